# revision 17
# baseline (speedup 1.0000x reference)
"""Trainium2 Bass kernel for nn_GaussianActor (moe_routing).

Strategy:
  - Data parallel over batch across 8 cores; weights replicated.
  - Host folds W3 into the per-stage heads (no activation between them):
      What[s] = W3 @ Wh[s],  bhat[s] = b3 @ Wh[s] + bh[s]
  - Host routes samples: each core gets 8 stage-segments of 512 columns
    (single-stage, so the head matmul weight is static) plus a 256-column
    overflow region where all 8 heads are computed and the host selects.
  - Device: feature-major activations (features on partitions, batch on
    free axis), fp32r matmuls, LayerNorm mean via a folded W0*ones/1024
    column, variance via Square + ones-vector matmul reduction.
  - Engine balance: bias-only evictions + squares on the vector engine,
    fused bias+LeakyReLU (Lrelu) evictions on the scalar engine.
"""

import numpy as np

import concourse.tile as tile
from concourse import bacc, mybir
from concourse import bass_utils
from concourse.alu_op_type import AluOpType

dt = mybir.dt
AF = mybir.ActivationFunctionType

B = 32768
OBS = 512
HID = 1024
A2 = 128          # 2 * action_dim
NSTAGE = 8
NCORES = 8
BC = B // NCORES  # 4096 samples per core

SEG = 512         # columns per stage segment
OVF = 256         # overflow columns per core
COLS = NSTAGE * SEG + OVF   # 4352 columns per core
NT_MAIN = NSTAGE  # 8 main tiles of width SEG (tile t -> stage t)

EPS = 1e-5
SLOPE = 0.01
LOG_STD_MIN, LOG_STD_MAX = -20.0, 2.0

KO = OBS // 128   # 4 k-blocks for layer 0
KH = HID // 128   # 8 k-blocks for hidden layers
MH = HID // 128   # 8 m-blocks of hidden features

_CACHE = {}


def _build_nc():
    nc = bacc.Bacc("TRN2", target_bir_lowering=False, debug=False,
                   num_devices=NCORES)

    obsT = nc.dram_tensor("obsT", [OBS, COLS], dt.float32r, kind="ExternalInput").ap()
    w0 = nc.dram_tensor("w0", [OBS, HID], dt.float32r, kind="ExternalInput").ap()
    w1 = nc.dram_tensor("w1", [HID, HID], dt.float32r, kind="ExternalInput").ap()
    w2 = nc.dram_tensor("w2", [HID, HID], dt.float32r, kind="ExternalInput").ap()
    wh = nc.dram_tensor("wh", [HID, NSTAGE * A2], dt.float32r, kind="ExternalInput").ap()
    wm = nc.dram_tensor("wm", [OBS, 1], dt.float32r, kind="ExternalInput").ap()
    b0d = nc.dram_tensor("b0d", [128, MH], dt.float32, kind="ExternalInput").ap()
    b1d = nc.dram_tensor("b1d", [128, MH], dt.float32, kind="ExternalInput").ap()
    b2d = nc.dram_tensor("b2d", [128, MH], dt.float32, kind="ExternalInput").ap()
    lnwd = nc.dram_tensor("lnwd", [128, MH], dt.float32, kind="ExternalInput").ap()
    lnbd = nc.dram_tensor("lnbd", [128, MH], dt.float32, kind="ExternalInput").ap()
    bhd = nc.dram_tensor("bhd", [128, NSTAGE], dt.float32, kind="ExternalInput").ap()
    mubd = nc.dram_tensor("mubd", [1, 1], dt.float32, kind="ExternalInput").ap()
    onesd = nc.dram_tensor("onesd", [128, 1], dt.float32r, kind="ExternalInput").ap()
    onesrd = nc.dram_tensor("onesrd", [1, 128], dt.float32r, kind="ExternalInput").ap()

    out_main = nc.dram_tensor("out_main", [A2, NSTAGE * SEG], dt.float32,
                              kind="ExternalOutput").ap()
    out_ovf = nc.dram_tensor("out_ovf", [NSTAGE * A2, OVF], dt.float32,
                             kind="ExternalOutput").ap()

    with tile.TileContext(nc) as tc:
        with tc.tile_pool(name="w", bufs=1) as wp, \
             tc.tile_pool(name="acts", bufs=1) as ap_, \
             tc.tile_pool(name="ps", bufs=6, space="PSUM") as pm, \
             tc.tile_pool(name="pbc", bufs=2, space="PSUM") as pbc:

            # ---- small constants first, then layer-0 weights ----
            _eng = [nc.sync, nc.scalar]
            b0t = wp.tile([128, MH], dt.float32, tag="b0t")
            nc.sync.dma_start(b0t[:], b0d[:])
            b1t = wp.tile([128, MH], dt.float32, tag="b1t")
            nc.sync.dma_start(b1t[:], b1d[:])
            b2t = wp.tile([128, MH], dt.float32, tag="b2t")
            nc.sync.dma_start(b2t[:], b2d[:])
            lnwt = wp.tile([128, MH], dt.float32, tag="lnwt")
            nc.sync.dma_start(lnwt[:], lnwd[:])
            lnbt = wp.tile([128, MH], dt.float32, tag="lnbt")
            nc.sync.dma_start(lnbt[:], lnbd[:])
            bht = wp.tile([128, NSTAGE], dt.float32, tag="bht")
            nc.sync.dma_start(bht[:], bhd[:])
            mubt = wp.tile([1, 1], dt.float32, tag="mubt")
            nc.sync.dma_start(mubt[:], mubd[:])
            onesk = wp.tile([128, 1], dt.float32r, tag="onesk")
            nc.sync.dma_start(onesk[:], onesd[:])
            onesr = wp.tile([1, 128], dt.float32r, tag="onesr")
            nc.sync.dma_start(onesr[:], onesrd[:])

            w0t = []
            for k in range(KO):
                t = wp.tile([128, HID], dt.float32r, tag=f"w0_{k}")
                _eng[k % 2].dma_start(t[:], w0[k * 128:(k + 1) * 128, :])
                w0t.append(t)
            wmt = wp.tile([128, KO], dt.float32r, tag="wm")
            for k in range(KO):
                nc.sync.dma_start(wmt[:, k:k + 1], wm[k * 128:(k + 1) * 128, :])

            w1t = w2t = wht = None

            def _load_deep_weights():
                a, b, c = [], [], []
                for k in range(KH):
                    t = wp.tile([128, HID], dt.float32r, tag=f"w1_{k}", name=f"w1_{k}")
                    nc.sync.dma_start(t[:], w1[k * 128:(k + 1) * 128, :])
                    a.append(t)
                for k in range(KH):
                    t = wp.tile([128, HID], dt.float32r, tag=f"w2_{k}", name=f"w2_{k}")
                    nc.sync.dma_start(t[:], w2[k * 128:(k + 1) * 128, :])
                    b.append(t)
                for k in range(KH):
                    t = wp.tile([128, NSTAGE * A2], dt.float32r, tag=f"wh_{k}",
                                name=f"wh_{k}")
                    nc.gpsimd.dma_start(t[:], wh[k * 128:(k + 1) * 128, :])
                    c.append(t)
                return a, b, c

            NTILES = NT_MAIN + 1

            def emit_l0(t):
                is_ovf = (t == NT_MAIN)
                tn = OVF if is_ovf else SEG
                c0 = t * SEG
                xk = []
                for k in range(KO):
                    xt = ap_.tile([128, tn], dt.float32r, tag="obsT", bufs=6,
                                  name=f"x_{t}_{k}")
                    nc.gpsimd.dma_start(xt[:], obsT[k * 128:(k + 1) * 128, c0:c0 + tn])
                    xk.append(xt)
                if t == 0:
                    st["w"] = _load_deep_weights()
                h0 = []
                for m in range(MH):
                    p = pm.tile([128, tn], dt.float32, tag="pm", bufs=6,
                                name=f"p0_{t}_{m}")
                    for k in range(KO):
                        nc.tensor.matmul(p[:], w0t[k][:, m * 128:(m + 1) * 128],
                                         xk[k][:], start=(k == 0), stop=(k == KO - 1))
                    h = ap_.tile([128, tn], dt.float32, tag="h0", bufs=10,
                                 name=f"h0_{t}_{m}")
                    nc.scalar.activation(h[:], p[:], AF.Identity,
                                         bias=b0t[:, m:m + 1], scale=1.0)
                    h0.append(h)
                pmu = pm.tile([1, tn], dt.float32, tag="pm", bufs=6, name=f"pmu_{t}")
                for k in range(KO):
                    nc.tensor.matmul(pmu[:], wmt[:, k:k + 1], xk[k][:],
                                     start=(k == 0), stop=(k == KO - 1))
                mu_f = ap_.tile([1, tn], dt.float32, tag="rows", bufs=3,
                                name=f"muf_{t}")
                nc.scalar.activation(mu_f[:], pmu[:], AF.Identity,
                                     bias=mubt[0:1, 0:1], scale=1.0)
                mu_r = ap_.tile([1, tn], dt.float32r, tag="rowsr", bufs=2,
                                name=f"mur_{t}")
                nc.scalar.copy(mu_r[:], mu_f[:])
                pM = pbc.tile([128, tn], dt.float32, tag="pbc", name=f"pM_{t}")
                nc.tensor.matmul(pM[:], onesr[:], mu_r[:], start=True, stop=True)
                return dict(t=t, tn=tn, c0=c0, is_ovf=is_ovf, h0=h0,
                            mu_f=mu_f, mu_r=mu_r, pM=pM)

            def emit_stats_bc(cur):
                t, tn, h0, mu_f, mu_r = cur["t"], cur["tn"], cur["h0"], cur["mu_f"], cur["mu_r"]
                pss = pm.tile([1, tn], dt.float32, tag="pm", bufs=6, name=f"pss_{t}")
                for m in range(MH):
                    sq = ap_.tile([128, tn], dt.float32r, tag="sq", bufs=2,
                                  name=f"sq_{t}_{m}")
                    nc.vector.tensor_tensor(sq[:], h0[m][:], h0[m][:], AluOpType.mult)
                    nc.tensor.matmul(pss[:], onesk[:], sq[:],
                                     start=(m == 0), stop=(m == MH - 1))
                ex2 = ap_.tile([1, tn], dt.float32, tag="rows", bufs=3, name=f"ex2_{t}")
                nc.scalar.mul(ex2[:], pss[:], 1.0 / HID)
                m2 = ap_.tile([1, tn], dt.float32, tag="rows", bufs=3, name=f"m2_{t}")
                nc.vector.tensor_tensor(m2[:], mu_f[:], mu_f[:], AluOpType.mult)
                var = ap_.tile([1, tn], dt.float32, tag="rows", bufs=3, name=f"var_{t}")
                nc.vector.tensor_tensor(var[:], ex2[:], m2[:], AluOpType.subtract)
                nc.vector.tensor_scalar_add(var[:], var[:], EPS)
                sd = ap_.tile([1, tn], dt.float32, tag="rows", bufs=3, name=f"sd_{t}")
                nc.scalar.activation(sd[:], var[:], AF.Sqrt, bias=0.0, scale=1.0)
                rstd_f = ap_.tile([1, tn], dt.float32, tag="rows", bufs=3,
                                  name=f"rsf_{t}")
                nc.vector.reciprocal(rstd_f[:], sd[:])
                rstd_r = ap_.tile([1, tn], dt.float32r, tag="rowsr", bufs=2,
                                  name=f"rsr_{t}")
                nc.scalar.copy(rstd_r[:], rstd_f[:])
                pR = pbc.tile([128, tn], dt.float32, tag="pbc", name=f"pR_{t}")
                nc.tensor.matmul(pR[:], onesr[:], rstd_r[:], start=True, stop=True)
                return cur["pM"], pR

            def emit_ln(cur, pM, pR):
                t, tn, h0 = cur["t"], cur["tn"], cur["h0"]
                h0n = []
                for m in range(MH):
                    c = ap_.tile([128, tn], dt.float32, tag="cd", bufs=6,
                                 name=f"c_{t}_{m}")
                    nc.vector.tensor_tensor(c[:], h0[m][:], pM[:], AluOpType.subtract)
                    nc.vector.tensor_tensor(c[:], c[:], pR[:], AluOpType.mult)
                    hn = ap_.tile([128, tn], dt.float32r, tag="hx", bufs=16,
                                  name=f"hn_{t}_{m}")
                    nc.scalar.activation(hn[:], c[:], AF.Lrelu,
                                         bias=lnbt[:, m:m + 1],
                                         scale=lnwt[:, m:m + 1], alpha=SLOPE)
                    h0n.append(hn)
                return h0n

            def emit_l123(cur, h0n):
                t, tn, c0, is_ovf = cur["t"], cur["tn"], cur["c0"], cur["is_ovf"]
                w1t, w2t, wht = st["w"]
                h1 = []
                for m in range(MH):
                    p = pm.tile([128, tn], dt.float32, tag="pm", bufs=6,
                                name=f"p1_{t}_{m}")
                    for k in range(KH):
                        nc.tensor.matmul(p[:], w1t[k][:, m * 128:(m + 1) * 128],
                                         h0n[k][:], start=(k == 0), stop=(k == KH - 1))
                    h = ap_.tile([128, tn], dt.float32r, tag="hx", bufs=16,
                                 name=f"h1_{t}_{m}")
                    nc.scalar.activation(h[:], p[:], AF.Lrelu,
                                         bias=b1t[:, m:m + 1], scale=1.0, alpha=SLOPE)
                    h1.append(h)
                h2 = []
                for m in range(MH):
                    p = pm.tile([128, tn], dt.float32, tag="pm", bufs=6,
                                name=f"p2_{t}_{m}")
                    for k in range(KH):
                        nc.tensor.matmul(p[:], w2t[k][:, m * 128:(m + 1) * 128],
                                         h1[k][:], start=(k == 0), stop=(k == KH - 1))
                    h = ap_.tile([128, tn], dt.float32r, tag="hx", bufs=16,
                                 name=f"h2_{t}_{m}")
                    nc.scalar.activation(h[:], p[:], AF.Lrelu,
                                         bias=b2t[:, m:m + 1], scale=1.0, alpha=SLOPE)
                    h2.append(h)
                heads = range(NSTAGE) if is_ovf else [t]
                for s_ in heads:
                    p = pm.tile([128, tn], dt.float32, tag="pm", bufs=6,
                                name=f"ph_{t}_{s_}")
                    for k in range(KH):
                        nc.tensor.matmul(p[:], wht[k][:, s_ * A2:(s_ + 1) * A2],
                                         h2[k][:], start=(k == 0), stop=(k == KH - 1))
                    o = ap_.tile([128, tn], dt.float32, tag="outp", bufs=2,
                                 name=f"o_{t}_{s_}")
                    nc.vector.tensor_scalar_add(o[:], p[:], bht[:, s_:s_ + 1])
                    if is_ovf:
                        nc.gpsimd.dma_start(out_ovf[s_ * A2:(s_ + 1) * A2, :], o[:])
                    else:
                        nc.gpsimd.dma_start(out_main[:, c0:c0 + tn], o[:])

            st = {}
            cur = emit_l0(0)
            cur_bc = emit_stats_bc(cur)
            for t in range(NTILES):
                h0n = emit_ln(cur, *cur_bc)
                if t + 1 < NTILES:
                    nxt = emit_l0(t + 1)
                    nxt_bc = emit_stats_bc(nxt)
                else:
                    nxt = nxt_bc = None
                emit_l123(cur, h0n)
                cur, cur_bc = nxt, nxt_bc

    nc.compile()
    return nc


def _get_nc():
    if "nc" not in _CACHE:
        _CACHE["nc"] = _build_nc()
    return _CACHE["nc"]


def _pack(stage):
    """Assign each sample to a (core, column). Returns perm [NCORES, COLS]
    (sample index per column; padded columns repeat sample 0) and
    valid [NCORES, COLS] bool."""
    perm = np.zeros((NCORES, COLS), np.int64)
    valid = np.zeros((NCORES, COLS), bool)
    overflow = []
    for s in range(NSTAGE):
        idx = np.where(stage == s)[0]
        cap = NCORES * SEG
        take = idx[:cap]
        overflow.extend(idx[cap:].tolist())
        for c in range(NCORES):
            seg = take[c * SEG:(c + 1) * SEG]
            if len(seg) == 0:
                continue
            cols = np.arange(s * SEG, s * SEG + len(seg))
            perm[c, cols] = seg
            valid[c, cols] = True
    if len(overflow) > NCORES * OVF:
        raise RuntimeError(f"overflow capacity exceeded: {len(overflow)}")
    for j, i in enumerate(overflow):
        c = j % NCORES
        col = NSTAGE * SEG + j // NCORES
        perm[c, col] = i
        valid[c, col] = True
    return perm, valid


def _prep(inputs):
    obs = np.asarray(inputs["obs"], np.float32)
    stage = np.asarray(inputs["stage"])
    W0 = np.asarray(inputs["W0"], np.float32)
    b0 = np.asarray(inputs["b0"], np.float32)
    ln_w = np.asarray(inputs["ln_w"], np.float32)
    ln_b = np.asarray(inputs["ln_b"], np.float32)
    W1 = np.asarray(inputs["W1"], np.float32)
    b1 = np.asarray(inputs["b1"], np.float32)
    W2 = np.asarray(inputs["W2"], np.float32)
    b2 = np.asarray(inputs["b2"], np.float32)
    W3 = np.asarray(inputs["W3"], np.float32)
    b3 = np.asarray(inputs["b3"], np.float32)
    Wh = np.asarray(inputs["Wh"], np.float32)
    bh = np.asarray(inputs["bh"], np.float32)

    # fold W3 into heads (fp64 for accuracy)
    What = np.einsum("kj,sjo->sko", W3.astype(np.float64), Wh.astype(np.float64))
    whcat = np.concatenate([What[s] for s in range(NSTAGE)], axis=1).astype(np.float32)
    bhat = (b3.astype(np.float64) @ Wh.astype(np.float64)
            + bh.astype(np.float64)).astype(np.float32)        # [S, A2]

    shared = {
        "w0": np.ascontiguousarray(W0),
        "w1": np.ascontiguousarray(W1),
        "w2": np.ascontiguousarray(W2),
        "wh": np.ascontiguousarray(whcat),
        "wm": np.ascontiguousarray(
            (W0.astype(np.float64).sum(axis=1) / HID).astype(np.float32)[:, None]),
        "b0d": np.ascontiguousarray(b0.reshape(MH, 128).T),
        "b1d": np.ascontiguousarray(b1.reshape(MH, 128).T),
        "b2d": np.ascontiguousarray(b2.reshape(MH, 128).T),
        "lnwd": np.ascontiguousarray(ln_w.reshape(MH, 128).T),
        "lnbd": np.ascontiguousarray(ln_b.reshape(MH, 128).T),
        "bhd": np.ascontiguousarray(bhat.T),
        "mubd": np.full((1, 1), float(b0.astype(np.float64).sum() / HID), np.float32),
        "onesd": np.ones((128, 1), np.float32),
        "onesrd": np.ones((1, 128), np.float32),
    }

    perm, valid = _pack(stage)
    in_maps = []
    for c in range(NCORES):
        m = dict(shared)
        m["obsT"] = np.ascontiguousarray(obs[perm[c]].T)
        in_maps.append(m)
    return in_maps, perm, valid, stage


def _unpack(results, perm, valid, stage):
    out = np.zeros((B, A2), np.float32)
    nmain = NSTAGE * SEG
    for c in range(NCORES):
        om = results[c]["out_main"]          # [A2, 4096]
        oo = results[c]["out_ovf"]           # [1024, OVF]
        vm = valid[c, :nmain]
        idx = perm[c, :nmain][vm]
        out[idx] = om[:, :nmain][:, vm].T
        vo = valid[c, nmain:]
        if vo.any():
            cols = np.where(vo)[0]
            iovf = perm[c, nmain:][vo]
            s = stage[iovf].astype(np.int64)
            oo3 = oo.reshape(NSTAGE, A2, OVF)
            out[iovf] = oo3[s, :, cols]
    return out


def _run(inputs, trace=False, tmpdir=None):
    nc = _get_nc()
    in_maps, perm, valid, stage = _prep(inputs)
    res = bass_utils.run_bass_kernel_spmd(nc, in_maps, list(range(NCORES)),
                                          trace=trace, tmpdir=tmpdir)
    out = _unpack(res.results, perm, valid, np.asarray(stage))
    mean = np.ascontiguousarray(out[:, :64])
    log_std = np.clip(out[:, 64:], LOG_STD_MIN, LOG_STD_MAX)
    return (mean, log_std), res


def kernel(**inputs):
    (mean, log_std), _ = _run(inputs, trace=False)
    return mean, log_std


def kernel_timed(_tmpdir=None, **inputs):
    (mean, log_std), res = _run(inputs, trace=True, tmpdir=_tmpdir)
    return (mean, log_std), res


# revision 18
# speedup vs baseline: 1.0126x; 1.0126x over previous
"""Trainium2 Bass kernel for nn_GaussianActor (moe_routing).

Strategy:
  - Data parallel over batch across 8 cores; weights replicated.
  - Host folds W3 into the per-stage heads (no activation between them):
      What[s] = W3 @ Wh[s],  bhat[s] = b3 @ Wh[s] + bh[s]
  - Host routes samples: each core gets 8 stage-segments of 512 columns
    (single-stage, so the head matmul weight is static) plus a 256-column
    overflow region where all 8 heads are computed and the host selects.
  - Device: feature-major activations (features on partitions, batch on
    free axis), fp32r matmuls, LayerNorm mean via a folded W0*ones/1024
    column, variance via Square + ones-vector matmul reduction.
  - Engine balance: bias-only evictions + squares on the vector engine,
    fused bias+LeakyReLU (Lrelu) evictions on the scalar engine.
"""

import numpy as np

import concourse.tile as tile
from concourse import bacc, mybir
from concourse import bass_utils
from concourse.alu_op_type import AluOpType

dt = mybir.dt
AF = mybir.ActivationFunctionType

B = 32768
OBS = 512
HID = 1024
A2 = 128          # 2 * action_dim
NSTAGE = 8
NCORES = 8
BC = B // NCORES  # 4096 samples per core

SEG = 512         # columns per stage segment
OVF = 256         # overflow columns per core
COLS = NSTAGE * SEG + OVF   # 4352 columns per core
NT_MAIN = NSTAGE  # 8 main tiles of width SEG (tile t -> stage t)

EPS = 1e-5
SLOPE = 0.01
LOG_STD_MIN, LOG_STD_MAX = -20.0, 2.0

KO = OBS // 128   # 4 k-blocks for layer 0
KH = HID // 128   # 8 k-blocks for hidden layers
MH = HID // 128   # 8 m-blocks of hidden features

_CACHE = {}


def _build_nc():
    nc = bacc.Bacc("TRN2", target_bir_lowering=False, debug=False,
                   num_devices=NCORES)

    obsT = nc.dram_tensor("obsT", [OBS, COLS], dt.float32r, kind="ExternalInput").ap()
    w0 = nc.dram_tensor("w0", [OBS, HID], dt.float32r, kind="ExternalInput").ap()
    w1 = nc.dram_tensor("w1", [HID, HID], dt.float32r, kind="ExternalInput").ap()
    w2 = nc.dram_tensor("w2", [HID, HID], dt.float32r, kind="ExternalInput").ap()
    wh = nc.dram_tensor("wh", [HID, NSTAGE * A2], dt.float32r, kind="ExternalInput").ap()
    wm = nc.dram_tensor("wm", [OBS, 1], dt.float32r, kind="ExternalInput").ap()
    b0d = nc.dram_tensor("b0d", [128, MH], dt.float32, kind="ExternalInput").ap()
    b1d = nc.dram_tensor("b1d", [128, MH], dt.float32, kind="ExternalInput").ap()
    b2d = nc.dram_tensor("b2d", [128, MH], dt.float32, kind="ExternalInput").ap()
    lnwd = nc.dram_tensor("lnwd", [128, MH], dt.float32, kind="ExternalInput").ap()
    lnbd = nc.dram_tensor("lnbd", [128, MH], dt.float32, kind="ExternalInput").ap()
    bhd = nc.dram_tensor("bhd", [128, NSTAGE], dt.float32, kind="ExternalInput").ap()
    mubd = nc.dram_tensor("mubd", [1, 1], dt.float32, kind="ExternalInput").ap()
    onesd = nc.dram_tensor("onesd", [128, 1], dt.float32r, kind="ExternalInput").ap()
    onesrd = nc.dram_tensor("onesrd", [1, 128], dt.float32r, kind="ExternalInput").ap()

    out_main = nc.dram_tensor("out_main", [A2, NSTAGE * SEG], dt.float32,
                              kind="ExternalOutput").ap()
    out_ovf = nc.dram_tensor("out_ovf", [NSTAGE * A2, OVF], dt.float32,
                             kind="ExternalOutput").ap()

    with tile.TileContext(nc) as tc:
        with tc.tile_pool(name="w", bufs=1) as wp, \
             tc.tile_pool(name="acts", bufs=1) as ap_, \
             tc.tile_pool(name="ps", bufs=6, space="PSUM") as pm, \
             tc.tile_pool(name="pbc", bufs=2, space="PSUM") as pbc:

            # ---- small constants first, then layer-0 weights ----
            _eng = [nc.sync, nc.gpsimd]
            b0t = wp.tile([128, MH], dt.float32, tag="b0t")
            nc.sync.dma_start(b0t[:], b0d[:])
            b1t = wp.tile([128, MH], dt.float32, tag="b1t")
            nc.sync.dma_start(b1t[:], b1d[:])
            b2t = wp.tile([128, MH], dt.float32, tag="b2t")
            nc.sync.dma_start(b2t[:], b2d[:])
            lnwt = wp.tile([128, MH], dt.float32, tag="lnwt")
            nc.sync.dma_start(lnwt[:], lnwd[:])
            lnbt = wp.tile([128, MH], dt.float32, tag="lnbt")
            nc.sync.dma_start(lnbt[:], lnbd[:])
            bht = wp.tile([128, NSTAGE], dt.float32, tag="bht")
            nc.sync.dma_start(bht[:], bhd[:])
            mubt = wp.tile([1, 1], dt.float32, tag="mubt")
            nc.sync.dma_start(mubt[:], mubd[:])
            onesk = wp.tile([128, 1], dt.float32r, tag="onesk")
            nc.sync.dma_start(onesk[:], onesd[:])
            onesr = wp.tile([1, 128], dt.float32r, tag="onesr")
            nc.sync.dma_start(onesr[:], onesrd[:])

            w0t = []
            for k in range(KO):
                t = wp.tile([128, HID], dt.float32r, tag=f"w0_{k}")
                _eng[k % 2].dma_start(t[:], w0[k * 128:(k + 1) * 128, :])
                w0t.append(t)
            wmt = wp.tile([128, KO], dt.float32r, tag="wm")
            for k in range(KO):
                nc.sync.dma_start(wmt[:, k:k + 1], wm[k * 128:(k + 1) * 128, :])

            w1t = w2t = wht = None

            def _load_deep_weights():
                a, b, c = [], [], []
                for k in range(KH):
                    t = wp.tile([128, HID], dt.float32r, tag=f"w1_{k}", name=f"w1_{k}")
                    nc.sync.dma_start(t[:], w1[k * 128:(k + 1) * 128, :])
                    a.append(t)
                for k in range(KH):
                    t = wp.tile([128, HID], dt.float32r, tag=f"w2_{k}", name=f"w2_{k}")
                    nc.sync.dma_start(t[:], w2[k * 128:(k + 1) * 128, :])
                    b.append(t)
                for k in range(KH):
                    t = wp.tile([128, NSTAGE * A2], dt.float32r, tag=f"wh_{k}",
                                name=f"wh_{k}")
                    nc.gpsimd.dma_start(t[:], wh[k * 128:(k + 1) * 128, :])
                    c.append(t)
                return a, b, c

            NTILES = NT_MAIN + 1

            def emit_l0(t):
                is_ovf = (t == NT_MAIN)
                tn = OVF if is_ovf else SEG
                c0 = t * SEG
                xk = []
                for k in range(KO):
                    xt = ap_.tile([128, tn], dt.float32r, tag="obsT", bufs=6,
                                  name=f"x_{t}_{k}")
                    nc.gpsimd.dma_start(xt[:], obsT[k * 128:(k + 1) * 128, c0:c0 + tn])
                    xk.append(xt)
                if t == 0:
                    st["w"] = _load_deep_weights()
                h0 = []
                for m in range(MH):
                    p = pm.tile([128, tn], dt.float32, tag="pm", bufs=6,
                                name=f"p0_{t}_{m}")
                    for k in range(KO):
                        nc.tensor.matmul(p[:], w0t[k][:, m * 128:(m + 1) * 128],
                                         xk[k][:], start=(k == 0), stop=(k == KO - 1))
                    h = ap_.tile([128, tn], dt.float32, tag="h0", bufs=10,
                                 name=f"h0_{t}_{m}")
                    nc.scalar.activation(h[:], p[:], AF.Identity,
                                         bias=b0t[:, m:m + 1], scale=1.0)
                    h0.append(h)
                pmu = pm.tile([1, tn], dt.float32, tag="pm", bufs=6, name=f"pmu_{t}")
                for k in range(KO):
                    nc.tensor.matmul(pmu[:], wmt[:, k:k + 1], xk[k][:],
                                     start=(k == 0), stop=(k == KO - 1))
                mu_f = ap_.tile([1, tn], dt.float32, tag="rows", bufs=3,
                                name=f"muf_{t}")
                nc.scalar.activation(mu_f[:], pmu[:], AF.Identity,
                                     bias=mubt[0:1, 0:1], scale=1.0)
                mu_r = ap_.tile([1, tn], dt.float32r, tag="rowsr", bufs=2,
                                name=f"mur_{t}")
                nc.scalar.copy(mu_r[:], mu_f[:])
                pM = pbc.tile([128, tn], dt.float32, tag="pbc", name=f"pM_{t}")
                nc.tensor.matmul(pM[:], onesr[:], mu_r[:], start=True, stop=True)
                return dict(t=t, tn=tn, c0=c0, is_ovf=is_ovf, h0=h0,
                            mu_f=mu_f, mu_r=mu_r, pM=pM)

            def emit_stats_bc(cur):
                t, tn, h0, mu_f, mu_r = cur["t"], cur["tn"], cur["h0"], cur["mu_f"], cur["mu_r"]
                pss = pm.tile([1, tn], dt.float32, tag="pm", bufs=6, name=f"pss_{t}")
                for m in range(MH):
                    sq = ap_.tile([128, tn], dt.float32r, tag="sq", bufs=2,
                                  name=f"sq_{t}_{m}")
                    nc.vector.tensor_tensor(sq[:], h0[m][:], h0[m][:], AluOpType.mult)
                    nc.tensor.matmul(pss[:], onesk[:], sq[:],
                                     start=(m == 0), stop=(m == MH - 1))
                ex2 = ap_.tile([1, tn], dt.float32, tag="rows", bufs=3, name=f"ex2_{t}")
                nc.scalar.mul(ex2[:], pss[:], 1.0 / HID)
                m2 = ap_.tile([1, tn], dt.float32, tag="rows", bufs=3, name=f"m2_{t}")
                nc.vector.tensor_tensor(m2[:], mu_f[:], mu_f[:], AluOpType.mult)
                var = ap_.tile([1, tn], dt.float32, tag="rows", bufs=3, name=f"var_{t}")
                nc.vector.tensor_tensor(var[:], ex2[:], m2[:], AluOpType.subtract)
                nc.vector.tensor_scalar_add(var[:], var[:], EPS)
                sd = ap_.tile([1, tn], dt.float32, tag="rows", bufs=3, name=f"sd_{t}")
                nc.scalar.activation(sd[:], var[:], AF.Sqrt, bias=0.0, scale=1.0)
                rstd_f = ap_.tile([1, tn], dt.float32, tag="rows", bufs=3,
                                  name=f"rsf_{t}")
                nc.vector.reciprocal(rstd_f[:], sd[:])
                rstd_r = ap_.tile([1, tn], dt.float32r, tag="rowsr", bufs=2,
                                  name=f"rsr_{t}")
                nc.scalar.copy(rstd_r[:], rstd_f[:])
                pR = pbc.tile([128, tn], dt.float32, tag="pbc", name=f"pR_{t}")
                nc.tensor.matmul(pR[:], onesr[:], rstd_r[:], start=True, stop=True)
                return cur["pM"], pR

            def emit_ln(cur, pM, pR):
                t, tn, h0 = cur["t"], cur["tn"], cur["h0"]
                h0n = []
                for m in range(MH):
                    c = ap_.tile([128, tn], dt.float32, tag="cd", bufs=6,
                                 name=f"c_{t}_{m}")
                    nc.vector.tensor_tensor(c[:], h0[m][:], pM[:], AluOpType.subtract)
                    nc.vector.tensor_tensor(c[:], c[:], pR[:], AluOpType.mult)
                    hn = ap_.tile([128, tn], dt.float32r, tag="hx", bufs=16,
                                  name=f"hn_{t}_{m}")
                    nc.scalar.activation(hn[:], c[:], AF.Lrelu,
                                         bias=lnbt[:, m:m + 1],
                                         scale=lnwt[:, m:m + 1], alpha=SLOPE)
                    h0n.append(hn)
                return h0n

            def emit_l123(cur, h0n):
                t, tn, c0, is_ovf = cur["t"], cur["tn"], cur["c0"], cur["is_ovf"]
                w1t, w2t, wht = st["w"]
                h1 = []
                for m in range(MH):
                    p = pm.tile([128, tn], dt.float32, tag="pm", bufs=6,
                                name=f"p1_{t}_{m}")
                    for k in range(KH):
                        nc.tensor.matmul(p[:], w1t[k][:, m * 128:(m + 1) * 128],
                                         h0n[k][:], start=(k == 0), stop=(k == KH - 1))
                    h = ap_.tile([128, tn], dt.float32r, tag="hx", bufs=16,
                                 name=f"h1_{t}_{m}")
                    nc.scalar.activation(h[:], p[:], AF.Lrelu,
                                         bias=b1t[:, m:m + 1], scale=1.0, alpha=SLOPE)
                    h1.append(h)
                h2 = []
                for m in range(MH):
                    p = pm.tile([128, tn], dt.float32, tag="pm", bufs=6,
                                name=f"p2_{t}_{m}")
                    for k in range(KH):
                        nc.tensor.matmul(p[:], w2t[k][:, m * 128:(m + 1) * 128],
                                         h1[k][:], start=(k == 0), stop=(k == KH - 1))
                    h = ap_.tile([128, tn], dt.float32r, tag="hx", bufs=16,
                                 name=f"h2_{t}_{m}")
                    nc.scalar.activation(h[:], p[:], AF.Lrelu,
                                         bias=b2t[:, m:m + 1], scale=1.0, alpha=SLOPE)
                    h2.append(h)
                heads = range(NSTAGE) if is_ovf else [t]
                for s_ in heads:
                    p = pm.tile([128, tn], dt.float32, tag="pm", bufs=6,
                                name=f"ph_{t}_{s_}")
                    for k in range(KH):
                        nc.tensor.matmul(p[:], wht[k][:, s_ * A2:(s_ + 1) * A2],
                                         h2[k][:], start=(k == 0), stop=(k == KH - 1))
                    o = ap_.tile([128, tn], dt.float32, tag="outp", bufs=2,
                                 name=f"o_{t}_{s_}")
                    nc.vector.tensor_scalar_add(o[:], p[:], bht[:, s_:s_ + 1])
                    if is_ovf:
                        nc.gpsimd.dma_start(out_ovf[s_ * A2:(s_ + 1) * A2, :], o[:])
                    else:
                        nc.gpsimd.dma_start(out_main[:, c0:c0 + tn], o[:])

            st = {}
            cur = emit_l0(0)
            cur_bc = emit_stats_bc(cur)
            for t in range(NTILES):
                h0n = emit_ln(cur, *cur_bc)
                if t + 1 < NTILES:
                    nxt = emit_l0(t + 1)
                    nxt_bc = emit_stats_bc(nxt)
                else:
                    nxt = nxt_bc = None
                emit_l123(cur, h0n)
                cur, cur_bc = nxt, nxt_bc

    nc.compile()
    return nc


def _get_nc():
    if "nc" not in _CACHE:
        _CACHE["nc"] = _build_nc()
    return _CACHE["nc"]


def _pack(stage):
    """Assign each sample to a (core, column). Returns perm [NCORES, COLS]
    (sample index per column; padded columns repeat sample 0) and
    valid [NCORES, COLS] bool."""
    perm = np.zeros((NCORES, COLS), np.int64)
    valid = np.zeros((NCORES, COLS), bool)
    overflow = []
    for s in range(NSTAGE):
        idx = np.where(stage == s)[0]
        cap = NCORES * SEG
        take = idx[:cap]
        overflow.extend(idx[cap:].tolist())
        for c in range(NCORES):
            seg = take[c * SEG:(c + 1) * SEG]
            if len(seg) == 0:
                continue
            cols = np.arange(s * SEG, s * SEG + len(seg))
            perm[c, cols] = seg
            valid[c, cols] = True
    if len(overflow) > NCORES * OVF:
        raise RuntimeError(f"overflow capacity exceeded: {len(overflow)}")
    for j, i in enumerate(overflow):
        c = j % NCORES
        col = NSTAGE * SEG + j // NCORES
        perm[c, col] = i
        valid[c, col] = True
    return perm, valid


def _prep(inputs):
    obs = np.asarray(inputs["obs"], np.float32)
    stage = np.asarray(inputs["stage"])
    W0 = np.asarray(inputs["W0"], np.float32)
    b0 = np.asarray(inputs["b0"], np.float32)
    ln_w = np.asarray(inputs["ln_w"], np.float32)
    ln_b = np.asarray(inputs["ln_b"], np.float32)
    W1 = np.asarray(inputs["W1"], np.float32)
    b1 = np.asarray(inputs["b1"], np.float32)
    W2 = np.asarray(inputs["W2"], np.float32)
    b2 = np.asarray(inputs["b2"], np.float32)
    W3 = np.asarray(inputs["W3"], np.float32)
    b3 = np.asarray(inputs["b3"], np.float32)
    Wh = np.asarray(inputs["Wh"], np.float32)
    bh = np.asarray(inputs["bh"], np.float32)

    # fold W3 into heads (fp64 for accuracy)
    What = np.einsum("kj,sjo->sko", W3.astype(np.float64), Wh.astype(np.float64))
    whcat = np.concatenate([What[s] for s in range(NSTAGE)], axis=1).astype(np.float32)
    bhat = (b3.astype(np.float64) @ Wh.astype(np.float64)
            + bh.astype(np.float64)).astype(np.float32)        # [S, A2]

    shared = {
        "w0": np.ascontiguousarray(W0),
        "w1": np.ascontiguousarray(W1),
        "w2": np.ascontiguousarray(W2),
        "wh": np.ascontiguousarray(whcat),
        "wm": np.ascontiguousarray(
            (W0.astype(np.float64).sum(axis=1) / HID).astype(np.float32)[:, None]),
        "b0d": np.ascontiguousarray(b0.reshape(MH, 128).T),
        "b1d": np.ascontiguousarray(b1.reshape(MH, 128).T),
        "b2d": np.ascontiguousarray(b2.reshape(MH, 128).T),
        "lnwd": np.ascontiguousarray(ln_w.reshape(MH, 128).T),
        "lnbd": np.ascontiguousarray(ln_b.reshape(MH, 128).T),
        "bhd": np.ascontiguousarray(bhat.T),
        "mubd": np.full((1, 1), float(b0.astype(np.float64).sum() / HID), np.float32),
        "onesd": np.ones((128, 1), np.float32),
        "onesrd": np.ones((1, 128), np.float32),
    }

    perm, valid = _pack(stage)
    in_maps = []
    for c in range(NCORES):
        m = dict(shared)
        m["obsT"] = np.ascontiguousarray(obs[perm[c]].T)
        in_maps.append(m)
    return in_maps, perm, valid, stage


def _unpack(results, perm, valid, stage):
    out = np.zeros((B, A2), np.float32)
    nmain = NSTAGE * SEG
    for c in range(NCORES):
        om = results[c]["out_main"]          # [A2, 4096]
        oo = results[c]["out_ovf"]           # [1024, OVF]
        vm = valid[c, :nmain]
        idx = perm[c, :nmain][vm]
        out[idx] = om[:, :nmain][:, vm].T
        vo = valid[c, nmain:]
        if vo.any():
            cols = np.where(vo)[0]
            iovf = perm[c, nmain:][vo]
            s = stage[iovf].astype(np.int64)
            oo3 = oo.reshape(NSTAGE, A2, OVF)
            out[iovf] = oo3[s, :, cols]
    return out


def _run(inputs, trace=False, tmpdir=None):
    nc = _get_nc()
    in_maps, perm, valid, stage = _prep(inputs)
    res = bass_utils.run_bass_kernel_spmd(nc, in_maps, list(range(NCORES)),
                                          trace=trace, tmpdir=tmpdir)
    out = _unpack(res.results, perm, valid, np.asarray(stage))
    mean = np.ascontiguousarray(out[:, :64])
    log_std = np.clip(out[:, 64:], LOG_STD_MIN, LOG_STD_MAX)
    return (mean, log_std), res


def kernel(**inputs):
    (mean, log_std), _ = _run(inputs, trace=False)
    return mean, log_std


def kernel_timed(_tmpdir=None, **inputs):
    (mean, log_std), res = _run(inputs, trace=True, tmpdir=_tmpdir)
    return (mean, log_std), res


# revision 19
# speedup vs baseline: 1.0251x; 1.0123x over previous
"""Trainium2 Bass kernel for nn_GaussianActor (moe_routing).

Strategy:
  - Data parallel over batch across 8 cores; weights replicated.
  - Host folds W3 into the per-stage heads (no activation between them):
      What[s] = W3 @ Wh[s],  bhat[s] = b3 @ Wh[s] + bh[s]
  - Host routes samples: each core gets 8 stage-segments of 512 columns
    (single-stage, so the head matmul weight is static) plus a 256-column
    overflow region where all 8 heads are computed and the host selects.
  - Device: feature-major activations (features on partitions, batch on
    free axis), fp32r matmuls, LayerNorm mean via a folded W0*ones/1024
    column, variance via Square + ones-vector matmul reduction.
  - Engine balance: bias-only evictions + squares on the vector engine,
    fused bias+LeakyReLU (Lrelu) evictions on the scalar engine.
"""

import numpy as np

import concourse.tile as tile
from concourse import bacc, mybir
from concourse import bass_utils
from concourse.alu_op_type import AluOpType

dt = mybir.dt
AF = mybir.ActivationFunctionType

B = 32768
OBS = 512
HID = 1024
A2 = 128          # 2 * action_dim
NSTAGE = 8
NCORES = 8
BC = B // NCORES  # 4096 samples per core

SEG = 512         # columns per stage segment
OVF = 256         # overflow columns per core
COLS = NSTAGE * SEG + OVF   # 4352 columns per core
NT_MAIN = NSTAGE  # 8 main tiles of width SEG (tile t -> stage t)

EPS = 1e-5
SLOPE = 0.01
LOG_STD_MIN, LOG_STD_MAX = -20.0, 2.0

KO = OBS // 128   # 4 k-blocks for layer 0
KH = HID // 128   # 8 k-blocks for hidden layers
MH = HID // 128   # 8 m-blocks of hidden features

_CACHE = {}


def _build_nc():
    nc = bacc.Bacc("TRN2", target_bir_lowering=False, debug=False,
                   num_devices=NCORES)

    obsT = nc.dram_tensor("obsT", [OBS, COLS], dt.float32r, kind="ExternalInput").ap()
    w0 = nc.dram_tensor("w0", [OBS, HID], dt.float32r, kind="ExternalInput").ap()
    w1 = nc.dram_tensor("w1", [HID, HID], dt.float32r, kind="ExternalInput").ap()
    w2 = nc.dram_tensor("w2", [HID, HID], dt.float32r, kind="ExternalInput").ap()
    wh = nc.dram_tensor("wh", [HID, NSTAGE * A2], dt.float32r, kind="ExternalInput").ap()
    wm = nc.dram_tensor("wm", [OBS, 1], dt.float32r, kind="ExternalInput").ap()
    b0d = nc.dram_tensor("b0d", [128, MH], dt.float32, kind="ExternalInput").ap()
    b1d = nc.dram_tensor("b1d", [128, MH], dt.float32, kind="ExternalInput").ap()
    b2d = nc.dram_tensor("b2d", [128, MH], dt.float32, kind="ExternalInput").ap()
    lnwd = nc.dram_tensor("lnwd", [128, MH], dt.float32, kind="ExternalInput").ap()
    lnbd = nc.dram_tensor("lnbd", [128, MH], dt.float32, kind="ExternalInput").ap()
    bhd = nc.dram_tensor("bhd", [128, NSTAGE], dt.float32, kind="ExternalInput").ap()
    mubd = nc.dram_tensor("mubd", [1, 1], dt.float32, kind="ExternalInput").ap()
    onesd = nc.dram_tensor("onesd", [128, 1], dt.float32r, kind="ExternalInput").ap()
    onesrd = nc.dram_tensor("onesrd", [1, 128], dt.float32r, kind="ExternalInput").ap()

    out_main = nc.dram_tensor("out_main", [A2, NSTAGE * SEG], dt.float32,
                              kind="ExternalOutput").ap()
    out_ovf = nc.dram_tensor("out_ovf", [NSTAGE * A2, OVF], dt.float32,
                             kind="ExternalOutput").ap()

    with tile.TileContext(nc) as tc:
        with tc.tile_pool(name="w", bufs=1) as wp, \
             tc.tile_pool(name="acts", bufs=1) as ap_, \
             tc.tile_pool(name="ps", bufs=6, space="PSUM") as pm, \
             tc.tile_pool(name="pbc", bufs=2, space="PSUM") as pbc:

            # ---- small constants first, then layer-0 weights ----
            _eng = [nc.sync, nc.gpsimd]
            b0t = wp.tile([128, MH], dt.float32, tag="b0t")
            nc.sync.dma_start(b0t[:], b0d[:])
            b1t = wp.tile([128, MH], dt.float32, tag="b1t")
            nc.sync.dma_start(b1t[:], b1d[:])
            b2t = wp.tile([128, MH], dt.float32, tag="b2t")
            nc.sync.dma_start(b2t[:], b2d[:])
            lnwt = wp.tile([128, MH], dt.float32, tag="lnwt")
            nc.sync.dma_start(lnwt[:], lnwd[:])
            lnbt = wp.tile([128, MH], dt.float32, tag="lnbt")
            nc.sync.dma_start(lnbt[:], lnbd[:])
            bht = wp.tile([128, NSTAGE], dt.float32, tag="bht")
            nc.sync.dma_start(bht[:], bhd[:])
            mubt = wp.tile([1, 1], dt.float32, tag="mubt")
            nc.sync.dma_start(mubt[:], mubd[:])
            onesk = wp.tile([128, 1], dt.float32r, tag="onesk")
            nc.sync.dma_start(onesk[:], onesd[:])
            onesr = wp.tile([1, 128], dt.float32r, tag="onesr")
            nc.sync.dma_start(onesr[:], onesrd[:])

            w0t = []
            for k in range(KO):
                t = wp.tile([128, HID], dt.float32r, tag=f"w0_{k}")
                _eng[k % 2].dma_start(t[:], w0[k * 128:(k + 1) * 128, :])
                w0t.append(t)
            wmt = wp.tile([128, KO], dt.float32r, tag="wm")
            for k in range(KO):
                nc.sync.dma_start(wmt[:, k:k + 1], wm[k * 128:(k + 1) * 128, :])

            w1t = w2t = wht = None

            def _load_deep_weights():
                a, b, c = [], [], []
                for k in range(KH):
                    t = wp.tile([128, HID], dt.float32r, tag=f"w1_{k}", name=f"w1_{k}")
                    nc.sync.dma_start(t[:], w1[k * 128:(k + 1) * 128, :])
                    a.append(t)
                for k in range(KH):
                    t = wp.tile([128, HID], dt.float32r, tag=f"w2_{k}", name=f"w2_{k}")
                    nc.sync.dma_start(t[:], w2[k * 128:(k + 1) * 128, :])
                    b.append(t)
                for k in range(KH):
                    t = wp.tile([128, NSTAGE * A2], dt.float32r, tag=f"wh_{k}",
                                name=f"wh_{k}")
                    nc.gpsimd.dma_start(t[:], wh[k * 128:(k + 1) * 128, :])
                    c.append(t)
                return a, b, c

            NTILES = NT_MAIN + 1

            def emit_l0(t):
                is_ovf = (t == NT_MAIN)
                tn = OVF if is_ovf else SEG
                c0 = t * SEG
                xk = []
                for k in range(KO):
                    xt = ap_.tile([128, tn], dt.float32r, tag="obsT", bufs=6,
                                  name=f"x_{t}_{k}")
                    nc.gpsimd.dma_start(xt[:], obsT[k * 128:(k + 1) * 128, c0:c0 + tn])
                    xk.append(xt)
                if t == 0:
                    st["w"] = _load_deep_weights()
                h0 = []
                for m in range(MH):
                    p = pm.tile([128, tn], dt.float32, tag="pm", bufs=6,
                                name=f"p0_{t}_{m}")
                    for k in range(KO):
                        nc.tensor.matmul(p[:], w0t[k][:, m * 128:(m + 1) * 128],
                                         xk[k][:], start=(k == 0), stop=(k == KO - 1))
                    h = ap_.tile([128, tn], dt.float32, tag="h0", bufs=10,
                                 name=f"h0_{t}_{m}")
                    nc.scalar.activation(h[:], p[:], AF.Identity,
                                         bias=b0t[:, m:m + 1], scale=1.0)
                    h0.append(h)
                pmu = pm.tile([1, tn], dt.float32, tag="pm", bufs=6, name=f"pmu_{t}")
                for k in range(KO):
                    nc.tensor.matmul(pmu[:], wmt[:, k:k + 1], xk[k][:],
                                     start=(k == 0), stop=(k == KO - 1))
                mu_f = ap_.tile([1, tn], dt.float32, tag="rows", bufs=3,
                                name=f"muf_{t}")
                nc.scalar.activation(mu_f[:], pmu[:], AF.Identity,
                                     bias=mubt[0:1, 0:1], scale=1.0)
                mu_r = ap_.tile([1, tn], dt.float32r, tag="rowsr", bufs=2,
                                name=f"mur_{t}")
                nc.scalar.copy(mu_r[:], mu_f[:])
                pM = pbc.tile([128, tn], dt.float32, tag="pbc", name=f"pM_{t}")
                nc.tensor.matmul(pM[:], onesr[:], mu_r[:], start=True, stop=True)
                return dict(t=t, tn=tn, c0=c0, is_ovf=is_ovf, h0=h0,
                            mu_f=mu_f, mu_r=mu_r, pM=pM)

            def emit_stats_bc(cur):
                t, tn, h0, mu_f, mu_r = cur["t"], cur["tn"], cur["h0"], cur["mu_f"], cur["mu_r"]
                pss = pm.tile([1, tn], dt.float32, tag="pm", bufs=6, name=f"pss_{t}")
                for m in range(MH):
                    sq = ap_.tile([128, tn], dt.float32r, tag="sq", bufs=2,
                                  name=f"sq_{t}_{m}")
                    nc.vector.tensor_tensor(sq[:], h0[m][:], h0[m][:], AluOpType.mult)
                    nc.tensor.matmul(pss[:], onesk[:], sq[:],
                                     start=(m == 0), stop=(m == MH - 1))
                ex2 = ap_.tile([1, tn], dt.float32, tag="rows", bufs=3, name=f"ex2_{t}")
                nc.scalar.mul(ex2[:], pss[:], 1.0 / HID)
                m2 = ap_.tile([1, tn], dt.float32, tag="rows", bufs=3, name=f"m2_{t}")
                nc.vector.tensor_tensor(m2[:], mu_f[:], mu_f[:], AluOpType.mult)
                var = ap_.tile([1, tn], dt.float32, tag="rows", bufs=3, name=f"var_{t}")
                nc.vector.tensor_tensor(var[:], ex2[:], m2[:], AluOpType.subtract)
                nc.vector.tensor_scalar_add(var[:], var[:], EPS)
                sd = ap_.tile([1, tn], dt.float32, tag="rows", bufs=3, name=f"sd_{t}")
                nc.scalar.activation(sd[:], var[:], AF.Sqrt, bias=0.0, scale=1.0)
                rstd_f = ap_.tile([1, tn], dt.float32, tag="rows", bufs=3,
                                  name=f"rsf_{t}")
                nc.vector.reciprocal(rstd_f[:], sd[:])
                rstd_r = ap_.tile([1, tn], dt.float32r, tag="rowsr", bufs=2,
                                  name=f"rsr_{t}")
                nc.scalar.copy(rstd_r[:], rstd_f[:])
                pR = pbc.tile([128, tn], dt.float32, tag="pbc", name=f"pR_{t}")
                nc.tensor.matmul(pR[:], onesr[:], rstd_r[:], start=True, stop=True)
                return cur["pM"], pR

            def emit_ln(cur, pM, pR):
                t, tn, h0 = cur["t"], cur["tn"], cur["h0"]
                h0n = []
                for m in range(MH):
                    c = ap_.tile([128, tn], dt.float32, tag="cd", bufs=6,
                                 name=f"c_{t}_{m}")
                    nc.vector.tensor_tensor(c[:], h0[m][:], pM[:], AluOpType.subtract)
                    nc.vector.tensor_tensor(c[:], c[:], pR[:], AluOpType.mult)
                    hn = ap_.tile([128, tn], dt.float32r, tag="hx", bufs=16,
                                  name=f"hn_{t}_{m}")
                    nc.scalar.activation(hn[:], c[:], AF.Lrelu,
                                         bias=lnbt[:, m:m + 1],
                                         scale=lnwt[:, m:m + 1], alpha=SLOPE)
                    h0n.append(hn)
                return h0n

            def emit_l123(cur, h0n):
                t, tn, c0, is_ovf = cur["t"], cur["tn"], cur["c0"], cur["is_ovf"]
                w1t, w2t, wht = st["w"]
                h1 = []
                for m in range(MH):
                    p = pm.tile([128, tn], dt.float32, tag="pm", bufs=6,
                                name=f"p1_{t}_{m}")
                    for k in range(KH):
                        nc.tensor.matmul(p[:], w1t[k][:, m * 128:(m + 1) * 128],
                                         h0n[k][:], start=(k == 0), stop=(k == KH - 1))
                    h = ap_.tile([128, tn], dt.float32r, tag="hx", bufs=16,
                                 name=f"h1_{t}_{m}")
                    nc.scalar.activation(h[:], p[:], AF.Lrelu,
                                         bias=b1t[:, m:m + 1], scale=1.0, alpha=SLOPE)
                    h1.append(h)
                h2 = []
                for m in range(MH):
                    p = pm.tile([128, tn], dt.float32, tag="pm", bufs=6,
                                name=f"p2_{t}_{m}")
                    for k in range(KH):
                        nc.tensor.matmul(p[:], w2t[k][:, m * 128:(m + 1) * 128],
                                         h1[k][:], start=(k == 0), stop=(k == KH - 1))
                    h = ap_.tile([128, tn], dt.float32r, tag="hx", bufs=16,
                                 name=f"h2_{t}_{m}")
                    nc.scalar.activation(h[:], p[:], AF.Lrelu,
                                         bias=b2t[:, m:m + 1], scale=1.0, alpha=SLOPE)
                    h2.append(h)
                heads = range(NSTAGE) if is_ovf else [t]
                for s_ in heads:
                    p = pm.tile([128, tn], dt.float32, tag="pm", bufs=6,
                                name=f"ph_{t}_{s_}")
                    for k in range(KH):
                        nc.tensor.matmul(p[:], wht[k][:, s_ * A2:(s_ + 1) * A2],
                                         h2[k][:], start=(k == 0), stop=(k == KH - 1))
                    o = ap_.tile([128, tn], dt.float32, tag="outp", bufs=2,
                                 name=f"o_{t}_{s_}")
                    nc.vector.tensor_scalar_add(o[:], p[:], bht[:, s_:s_ + 1])
                    if is_ovf:
                        nc.gpsimd.dma_start(out_ovf[s_ * A2:(s_ + 1) * A2, :], o[:])
                    else:
                        nc.gpsimd.dma_start(out_main[:, c0:c0 + tn], o[:])

            st = {}
            order = [0, 1, 2, 3, 4, 5, 6, NT_MAIN, 7]
            cur = emit_l0(order[0])
            cur_bc = emit_stats_bc(cur)
            for i in range(len(order)):
                h0n = emit_ln(cur, *cur_bc)
                if i + 1 < len(order):
                    nxt = emit_l0(order[i + 1])
                    nxt_bc = emit_stats_bc(nxt)
                else:
                    nxt = nxt_bc = None
                emit_l123(cur, h0n)
                cur, cur_bc = nxt, nxt_bc

    nc.compile()
    return nc


def _get_nc():
    if "nc" not in _CACHE:
        _CACHE["nc"] = _build_nc()
    return _CACHE["nc"]


def _pack(stage):
    """Assign each sample to a (core, column). Returns perm [NCORES, COLS]
    (sample index per column; padded columns repeat sample 0) and
    valid [NCORES, COLS] bool."""
    perm = np.zeros((NCORES, COLS), np.int64)
    valid = np.zeros((NCORES, COLS), bool)
    overflow = []
    for s in range(NSTAGE):
        idx = np.where(stage == s)[0]
        cap = NCORES * SEG
        take = idx[:cap]
        overflow.extend(idx[cap:].tolist())
        for c in range(NCORES):
            seg = take[c * SEG:(c + 1) * SEG]
            if len(seg) == 0:
                continue
            cols = np.arange(s * SEG, s * SEG + len(seg))
            perm[c, cols] = seg
            valid[c, cols] = True
    if len(overflow) > NCORES * OVF:
        raise RuntimeError(f"overflow capacity exceeded: {len(overflow)}")
    for j, i in enumerate(overflow):
        c = j % NCORES
        col = NSTAGE * SEG + j // NCORES
        perm[c, col] = i
        valid[c, col] = True
    return perm, valid


def _prep(inputs):
    obs = np.asarray(inputs["obs"], np.float32)
    stage = np.asarray(inputs["stage"])
    W0 = np.asarray(inputs["W0"], np.float32)
    b0 = np.asarray(inputs["b0"], np.float32)
    ln_w = np.asarray(inputs["ln_w"], np.float32)
    ln_b = np.asarray(inputs["ln_b"], np.float32)
    W1 = np.asarray(inputs["W1"], np.float32)
    b1 = np.asarray(inputs["b1"], np.float32)
    W2 = np.asarray(inputs["W2"], np.float32)
    b2 = np.asarray(inputs["b2"], np.float32)
    W3 = np.asarray(inputs["W3"], np.float32)
    b3 = np.asarray(inputs["b3"], np.float32)
    Wh = np.asarray(inputs["Wh"], np.float32)
    bh = np.asarray(inputs["bh"], np.float32)

    # fold W3 into heads (fp64 for accuracy)
    What = np.einsum("kj,sjo->sko", W3.astype(np.float64), Wh.astype(np.float64))
    whcat = np.concatenate([What[s] for s in range(NSTAGE)], axis=1).astype(np.float32)
    bhat = (b3.astype(np.float64) @ Wh.astype(np.float64)
            + bh.astype(np.float64)).astype(np.float32)        # [S, A2]

    shared = {
        "w0": np.ascontiguousarray(W0),
        "w1": np.ascontiguousarray(W1),
        "w2": np.ascontiguousarray(W2),
        "wh": np.ascontiguousarray(whcat),
        "wm": np.ascontiguousarray(
            (W0.astype(np.float64).sum(axis=1) / HID).astype(np.float32)[:, None]),
        "b0d": np.ascontiguousarray(b0.reshape(MH, 128).T),
        "b1d": np.ascontiguousarray(b1.reshape(MH, 128).T),
        "b2d": np.ascontiguousarray(b2.reshape(MH, 128).T),
        "lnwd": np.ascontiguousarray(ln_w.reshape(MH, 128).T),
        "lnbd": np.ascontiguousarray(ln_b.reshape(MH, 128).T),
        "bhd": np.ascontiguousarray(bhat.T),
        "mubd": np.full((1, 1), float(b0.astype(np.float64).sum() / HID), np.float32),
        "onesd": np.ones((128, 1), np.float32),
        "onesrd": np.ones((1, 128), np.float32),
    }

    perm, valid = _pack(stage)
    in_maps = []
    for c in range(NCORES):
        m = dict(shared)
        m["obsT"] = np.ascontiguousarray(obs[perm[c]].T)
        in_maps.append(m)
    return in_maps, perm, valid, stage


def _unpack(results, perm, valid, stage):
    out = np.zeros((B, A2), np.float32)
    nmain = NSTAGE * SEG
    for c in range(NCORES):
        om = results[c]["out_main"]          # [A2, 4096]
        oo = results[c]["out_ovf"]           # [1024, OVF]
        vm = valid[c, :nmain]
        idx = perm[c, :nmain][vm]
        out[idx] = om[:, :nmain][:, vm].T
        vo = valid[c, nmain:]
        if vo.any():
            cols = np.where(vo)[0]
            iovf = perm[c, nmain:][vo]
            s = stage[iovf].astype(np.int64)
            oo3 = oo.reshape(NSTAGE, A2, OVF)
            out[iovf] = oo3[s, :, cols]
    return out


def _run(inputs, trace=False, tmpdir=None):
    nc = _get_nc()
    in_maps, perm, valid, stage = _prep(inputs)
    res = bass_utils.run_bass_kernel_spmd(nc, in_maps, list(range(NCORES)),
                                          trace=trace, tmpdir=tmpdir)
    out = _unpack(res.results, perm, valid, np.asarray(stage))
    mean = np.ascontiguousarray(out[:, :64])
    log_std = np.clip(out[:, 64:], LOG_STD_MIN, LOG_STD_MAX)
    return (mean, log_std), res


def kernel(**inputs):
    (mean, log_std), _ = _run(inputs, trace=False)
    return mean, log_std


def kernel_timed(_tmpdir=None, **inputs):
    (mean, log_std), res = _run(inputs, trace=True, tmpdir=_tmpdir)
    return (mean, log_std), res
